# revision 2
# baseline (speedup 1.0000x reference)
"""Trainium2 Bass kernel for spatial self-attention (B=4, C=64, H=W=64,
4 heads x 4 dim), N-sharded: 512 query positions per core; the host
unshard is a concatenate along the flattened spatial axis.

Pipeline per core:
  - QK sim matmuls emitted as row-tiled pairs (tile_position (0,0)/(64,0),
    K=64 each) so consecutive MMs stream concurrently on HW; kq is
    duplicated into partitions 64-127 to feed the upper row tiles.
  - sim PSUM is a 6-bank ring of [128,512] slots; softmax exp runs per
    3-slot chunk [128,1536] and is load-balanced between ScalarE (exact
    exp, PSUM->SBUF bf16 direct) and VectorE (Schraudolph bitcast exp:
    e = bf16_bitcast(int16(sim*184.665 + 16250)), one tensor_scalar op).
  - AV matmuls col-tiled across the 4 heads (tile_position (0,32h)) and
    accumulated directly in one PSUM bank across all 32 key tiles
    (start=jt==0), eliminating per-tile DVE folds.
  - the DRAM x tensor carries a pre-baked ones row (row 64) so the
    [ones|v] stationary trick needs no 1-partition memsets.
"""

import os
import sys

for p in ("/opt/trn_rl_repo", "/opt/pypackages"):
    if p not in sys.path:
        sys.path.insert(0, p)

os.environ.setdefault("MYCRO_LOCAL_CACHE", "1")

import numpy as np

import concourse.bass as bass
import concourse.mybir as mybir
import concourse.tile as tile
from concourse import bacc
from concourse.bass_utils import run_bass_kernel_spmd
from concourse import bass2jax as _b2j

# --- NEFF cache: walrus compiles of the same HLO/BIR are cached on disk ---
_NEFF_CACHE_DIR = "/root/neff_cache"
_orig_hook = _b2j.neuronx_cc_hook


def _caching_neuronx_cc_hook(code, code_format, platform_version, file_prefix):
    import hashlib

    key = hashlib.sha256(
        bytes(code) + bytes(code_format) + str(platform_version).encode()
    ).hexdigest()
    path = os.path.join(_NEFF_CACHE_DIR, key + ".bin")
    if os.path.exists(path):
        with open(path, "rb") as f:
            return 0, f.read()
    r, data = _orig_hook(code, code_format, platform_version, file_prefix)
    try:
        os.makedirs(_NEFF_CACHE_DIR, exist_ok=True)
        tmp = path + ".tmp"
        with open(tmp, "wb") as f:
            f.write(data)
        os.replace(tmp, path)
    except Exception:
        pass
    return r, data


_b2j.neuronx_cc_hook = _caching_neuronx_cc_hook

BF16 = mybir.dt.bfloat16
F32 = mybir.dt.float32
I16 = mybir.dt.int16
NPB = mybir.dt.np(BF16)

B = 4
C = 64
HW = 64
N = HW * HW  # 4096
HEADS = 4
DH = 4
SCALE = DH**-0.5
NCORES = 8
IS = N // NCORES  # 512
JT = N // 128  # 32

# Schraudolph bf16 exp constants (trunc-toward-zero semantics; u>0 always)
A_SCH = 184.6650
B_SCH = 16250.0

# greedy chunk balance: modelled per-chunk cost (ns) and DVE preload for
# its copy/tail duties
ACT_CHUNK_NS = 1055.0
DVE_CHUNK_NS = 1265.0
DVE_PRELOAD_NS = 50000.0
ACT_PRELOAD_NS = 2700.0
AV_LAG = 6
DVE_SHARE_NUM = 4
DVE_SHARE_DEN = 9
SPC = 2  # slots per exp chunk
CHW = SPC * 512  # chunk width
ETS_BUFS = 8


def build_graph():
    nc = bacc.Bacc(
        "TRN2", target_bir_lowering=False, debug=False, num_devices=NCORES
    )

    xa_ext = nc.dram_tensor("xa", [B, C + 1, N], BF16, kind="ExternalInput").ap()
    xq_ext = nc.dram_tensor("xq", [B, C, IS], BF16, kind="ExternalInput").ap()
    wq_ext = nc.dram_tensor("wq_sp", [C, 128], BF16, kind="ExternalInput").ap()
    wk_ext = nc.dram_tensor("wk_sp", [128, C], BF16, kind="ExternalInput").ap()
    wv_ext = nc.dram_tensor("wv_pad", [C + 1, 20], BF16, kind="ExternalInput").ap()
    wo_ext = nc.dram_tensor("wo_sp", [128, C], BF16, kind="ExternalInput").ap()
    bias_ext = nc.dram_tensor("b_out", [C, 1], F32, kind="ExternalInput").ap()
    out_ext = nc.dram_tensor("out", [B, C, IS], F32, kind="ExternalOutput").ap()

    with tile.TileContext(nc) as tc:
        with (
            tc.tile_pool(name="const", bufs=1) as cst,
            tc.tile_pool(name="big", bufs=1) as big,
            tc.tile_pool(name="expa", bufs=4) as expa,
            tc.tile_pool(name="expd", bufs=6) as expd,
            tc.tile_pool(name="psum", bufs=1, space="PSUM") as psump,
        ):
            wq_s = cst.tile([C, 128], BF16, tag="wq", name="wq_s")
            wk_s = cst.tile([128, C], BF16, tag="wk", name="wk_s")
            wv_s = cst.tile([C + 1, 20], BF16, tag="wv", name="wv_s")
            wo_s = cst.tile([128, C], BF16, tag="wo", name="wo_s")
            bias_s = cst.tile([C, 1], F32, tag="bias", name="bias_s")
            ones_bf = cst.tile([128, 5], BF16, tag="onesb", name="ones_bf")
            rec_t = cst.tile([128, IS], F32, tag="rec", name="rec_t")
            accb = cst.tile([128, IS], BF16, tag="accb", name="accb")
            nc.sync.dma_start(out=wq_s[:], in_=wq_ext)
            nc.sync.dma_start(out=wk_s[:], in_=wk_ext)
            nc.sync.dma_start(out=wv_s[:], in_=wv_ext)
            nc.sync.dma_start(out=wo_s[:], in_=wo_ext)
            nc.sync.dma_start(out=bias_s[:], in_=bias_ext)
            nc.vector.memset(ones_bf[:], 1.0)

            xs = [
                big.tile([C + 1, N], BF16, tag=f"xs{b}", name=f"xs{b}")
                for b in range(B)
            ]
            xd = [
                big.tile([128, N], BF16, tag=f"xd{b}", name=f"xd{b}")
                for b in range(B)
            ]
            kq2 = [
                big.tile([128, 4 * IS], BF16, tag=f"kq{b}", name=f"kq{b}")
                for b in range(B)
            ]
            vT = [
                big.tile([128, 20 * JT], BF16, tag=f"vT{b}", name=f"vT{b}")
                for b in range(B)
            ]
            qs = [
                big.tile([128, IS], BF16, tag=f"qs{b}", name=f"qs{b}")
                for b in range(B)
            ]
            xqs = [
                big.tile([C, IS], BF16, tag=f"xqs{b}", name=f"xqs{b}")
                for b in range(B)
            ]
            acc = [
                big.tile([128, IS], F32, tag=f"acc{b}", name=f"acc{b}")
                for b in range(B)
            ]
            att = [
                big.tile([128, IS], BF16, tag=f"att{b}", name=f"att{b}")
                for b in range(B)
            ]
            ys = [
                big.tile([C, IS], F32, tag=f"ys{b}", name=f"ys{b}")
                for b in range(B)
            ]

            for b in range(B):
                nc.sync.dma_start(out=xs[b][:], in_=xa_ext[b])
                nc.sync.dma_start(out=xd[b][64:128, :], in_=xa_ext[b][0:64, :])
                nc.sync.dma_start(out=xqs[b][:], in_=xq_ext[b])
                nc.vector.memset(att[b][:], 0.0)

            wa = [
                psump.tile([128, 1024], F32, tag=f"wa{i}", name=f"wa{i}")
                for i in range(2)
            ]
            wd = [
                psump.tile([128, 512], F32, tag=f"wd{i}", name=f"wd{i}")
                for i in range(3)
            ]
            avb = psump.tile([128, IS], F32, tag="avb", name="avb")

            # seed every avb partition once so tail reads are fully
            # initialized (junk rows flow to never-read acc rows)
            zw_t = cst.tile([C, 128], BF16, tag="zw", name="zw_t")
            nc.vector.memset(zw_t[:], 0.0)
            nc.tensor.matmul(
                avb[:, :],
                zw_t[:],
                xqs[0][:],
                start=True,
                stop=True,
                skip_group_check=True,
            )

            # ---------- slot/chunk machinery ----------
            state = {
                "S": 0,  # next slot index
                "qk_slots": {},  # chunk -> set of filled positions from QK
                "filled": {},  # chunk -> count of written slots
                "ets": {},  # chunk -> (tile, engine)
                "act_t": ACT_PRELOAD_NS,
                "dve_t": DVE_PRELOAD_NS,
                "done_chunk": -1,
                "processed": -1,
                "pending_av": [],
                "n_act": 0,
                "n_dve": 0,
            }

            # slot pattern period = 3 slots: slots 3p,3p+1 form ACT chunk
            # A_p ([128,1024], 2 windows over banks 0-3); slot 3p+2 is DVE
            # chunk D_p ([128,512], 3 windows over banks 4-6). Chunk seq id:
            # A_p = 2p, D_p = 2p+1. Deep per-family rotation keeps the
            # in-order PE from waiting on any in-flight exp (window WAR
            # lands >=2 family-chunks back).
            def chunk_of_slot(s):
                p, r = divmod(s, 3)
                return 2 * p + (1 if r == 2 else 0)

            def last_slot_of(kc):
                p, d = divmod(kc, 2)
                return 3 * p + (2 if d else 1)

            def slot_ap_at(s):
                p, r = divmod(s, 3)
                if r < 2:
                    return wa[p % 2][:, r * 512 : (r + 1) * 512]
                return wd[p % 3][:, :]

            def slot_ap():
                return slot_ap_at(state["S"])

            def chunk_window(kc, width):
                p, d = divmod(kc, 2)
                if d == 0:
                    return wa[p % 2][:, 0:width]
                return wd[p % 3][:, 0:width]

            def process_chunk(k):
                """Emit exp (ACT) or Schraudolph (DVE) for chunk k if it has
                QK slots; then release any AV groups waiting on it."""
                nfill = state["filled"].get(k, 0)
                has_qk = bool(state["qk_slots"].get(k))
                if has_qk:
                    width = nfill * 512
                    on_act = (k % 2) == 0
                    if on_act:
                        et = expa.tile(
                            [128, 1024], BF16, tag="etsa", name=f"ets{k}",
                            bufs=4,
                        )
                        state["n_act"] += 1
                        nc.scalar.activation(
                            et[:, 0:width],
                            chunk_window(k, width),
                            mybir.ActivationFunctionType.Exp,
                        )
                    else:
                        et = expd.tile(
                            [128, 512], BF16, tag="etsd", name=f"ets{k}",
                            bufs=6,
                        )
                        state["n_dve"] += 1
                        nc.vector.tensor_scalar(
                            et[:, 0:width].bitcast(I16),
                            chunk_window(k, width),
                            A_SCH,
                            B_SCH,
                            mybir.AluOpType.mult,
                            mybir.AluOpType.add,
                        )
                    state["ets"][k] = et
                state["done_chunk"] = k
                lag = 0 if state.get("force") else AV_LAG
                still = []
                for b, jt, slots in state["pending_av"]:
                    if max(chunk_of_slot(s) for s in slots) + lag <= k:
                        emit_av(b, jt, slots)
                    else:
                        still.append((b, jt, slots))
                state["pending_av"] = still

            def advance(qk, n=1):
                for _ in range(n):
                    k = chunk_of_slot(state["S"])
                    state["filled"][k] = state["filled"].get(k, 0) + 1
                    if qk:
                        state["qk_slots"].setdefault(k, set()).add(
                            state["S"] % 3 if state["S"] % 3 < 2 else 0
                        )
                    state["S"] += 1

            def drain():
                # process every fully-emitted chunk, in order; tail()/emit_av
                # may advance S further while we loop
                while last_slot_of(state["processed"] + 1) < state["S"]:
                    k = state["processed"] + 1
                    state["processed"] = k
                    process_chunk(k)

            def pad_to_chunk():
                # skip unwritten slots so the next emission starts a fresh
                # 3-slot period; an untouched pad slot is never read
                if state["S"] % 3:
                    state["S"] = (state["S"] // 3 + 1) * 3
                drain()

            def emit_av(b, jt, slots):
                for h in range(HEADS):
                    s = slots[h]
                    et = state["ets"][chunk_of_slot(s)]
                    pos = s % 3 if s % 3 < 2 else 0
                    src = et[:, pos * 512 : pos * 512 + 512]
                    nc.tensor.matmul(
                        avb[32 * h : 32 * h + 5, :],
                        vT[b][:, 20 * jt + 5 * h : 20 * jt + 5 * h + 5],
                        src,
                        start=(jt == 0),
                        stop=(jt == JT - 1),
                        tile_position=(0, 32 * h),
                        skip_group_check=True,
                    )
                if jt == JT - 1:
                    tail(b)

            def tail(b):
                # fold the AV bank out, normalize, project, bias, store
                nc.vector.tensor_copy(acc[b][:], avb[:])
                for h in range(HEADS):
                    nc.vector.reciprocal(
                        rec_t[32 * h : 32 * h + 1, :],
                        acc[b][32 * h : 32 * h + 1, :],
                    )
                for h in range(HEADS):
                    nc.vector.tensor_copy(
                        accb[32 * h : 32 * h + 1, :],
                        rec_t[32 * h : 32 * h + 1, :],
                    )
                sx = slot_ap()
                for h in range(HEADS):
                    nc.tensor.matmul(
                        sx[32 * h : 32 * h + 5, :],
                        ones_bf[32 * h : 32 * h + 1, 0:5],
                        accb[32 * h : 32 * h + 1, :],
                        start=True,
                        stop=True,
                        tile_position=(32 * h, 32 * h),
                        skip_group_check=True,
                    )
                advance(qk=False)
                for h in range(HEADS):
                    nc.vector.tensor_tensor(
                        att[b][32 * h : 32 * h + 5, :],
                        acc[b][32 * h : 32 * h + 5, :],
                        sx[32 * h : 32 * h + 5, :],
                        mybir.AluOpType.mult,
                    )
                sy = slot_ap()
                nc.tensor.matmul(
                    sy[0:C, :], wo_s[:], att[b][:], start=True, stop=True
                )
                advance(qk=False)
                nc.vector.tensor_scalar(
                    ys[b][:],
                    sy[0:C, :],
                    bias_s[:],
                    None,
                    mybir.AluOpType.add,
                )
                nc.sync.dma_start(out=out_ext[b], in_=ys[b][:])

            def pieces(b):
                pad_to_chunk()
                # q projection -> qs
                sq = slot_ap()
                nc.tensor.matmul(
                    sq[:, :], wq_s[:], xqs[b][:], start=True, stop=True
                )
                advance(qk=False)
                nc.vector.tensor_copy(qs[b][:], sq[:, :])
                # kq = wk^T q, duplicated into partitions 64-127
                for h in range(HEADS):
                    sk = slot_ap()
                    nc.tensor.matmul(
                        sk[0:C, :],
                        wk_s[32 * h : 32 * h + DH, :],
                        qs[b][32 * h : 32 * h + DH, :],
                        start=True,
                        stop=True,
                        tile_position=(32 * h, 0),
                        skip_group_check=True,
                    )
                    nc.tensor.matmul(
                        sk[64:128, :],
                        wk_s[32 * h : 32 * h + DH, :],
                        qs[b][32 * h : 32 * h + DH, :],
                        start=True,
                        stop=True,
                        tile_position=(32 * h, 64),
                        skip_group_check=True,
                    )
                    advance(qk=False)
                    nc.vector.tensor_copy(
                        kq2[b][:, 512 * h : 512 * (h + 1)], sk[:, :]
                    )
                # vT pieces: 8 groups of 4 key tiles, 6 groups per slot
                svA = slot_ap_at(state["S"])
                svB = slot_ap_at(state["S"] + 1)
                for g in range(8):
                    sv = svA if g < 6 else svB
                    for k4 in range(4):
                        jt = 4 * g + k4
                        col = 80 * (g % 6) + 20 * k4
                        first = k4 == 0 and g % 6 == 0
                        last = (g % 6 == 5 or g == 7) and k4 == 3
                        nc.tensor.matmul(
                            sv[:, col : col + 20],
                            xs[b][:, jt * 128 : (jt + 1) * 128],
                            wv_s[:],
                            start=first,
                            stop=last,
                        )
                nc.vector.tensor_copy(vT[b][:, 0:480], svA[:, 0:480])
                nc.vector.tensor_copy(vT[b][:, 480:640], svB[:, 0:160])
                advance(qk=False, n=2)
                pad_to_chunk()

            # ---------- main schedule ----------
            pieces(0)
            for b in range(B):
                for jt in range(JT):
                    s0 = state["S"]
                    slots = [s0, s0 + 1, s0 + 2, s0 + 3]
                    for hp in range(2):
                        hlo, hhi = 2 * hp, 2 * hp + 1
                        sa = slot_ap_at(s0 + 2 * hp)
                        sb_ = slot_ap_at(s0 + 2 * hp + 1)
                        nc.tensor.matmul(
                            sa[:, :],
                            xs[b][0:C, jt * 128 : (jt + 1) * 128],
                            kq2[b][0:C, 512 * hlo : 512 * (hlo + 1)],
                            start=True,
                            stop=True,
                            tile_position=(0, 0),
                        )
                        nc.tensor.matmul(
                            sb_[:, :],
                            xd[b][64:128, jt * 128 : (jt + 1) * 128],
                            kq2[b][64:128, 512 * hhi : 512 * (hhi + 1)],
                            start=True,
                            stop=True,
                            tile_position=(64, 0),
                        )
                        advance(qk=True, n=2)
                    # heads emitted at slots s0..s0+3 map h order [0,1,2,3]
                    need = max(chunk_of_slot(s) for s in slots) + AV_LAG
                    if need <= state["done_chunk"]:
                        emit_av(b, jt, slots)
                    else:
                        state["pending_av"].append((b, jt, slots))
                    drain()
                    if jt == 10 and b + 1 < B:
                        pieces(b + 1)
            # flush a possibly-partial final chunk and drain AV
            state["force"] = True
            pad_to_chunk()
            if state["pending_av"]:
                rest = state["pending_av"]
                state["pending_av"] = []
                for b_, jt_, slots_ in rest:
                    emit_av(b_, jt_, slots_)
            assert not state["pending_av"]

    nc.compile()
    print(
        f"[kernel2] exp chunks: ACT={state['n_act']} DVE={state['n_dve']}"
    )
    return nc


def host_prep(x, w_qkv, w_out, b_out):
    x3 = np.ascontiguousarray(x.reshape(B, C, N), dtype=np.float32)
    xa = np.ones((B, C + 1, N), np.float32)
    xa[:, 0:C, :] = x3
    xa_bf = xa.astype(NPB)
    wq = w_qkv[0:16].astype(np.float32) * SCALE
    wk = w_qkv[16:32].astype(np.float32)
    wv = w_qkv[32:48].astype(np.float32)

    wq_sp = np.zeros((C, 128), np.float32)
    wk_sp = np.zeros((128, C), np.float32)
    for h in range(HEADS):
        for d in range(DH):
            wq_sp[:, 32 * h + d] = wq[4 * h + d]
            wk_sp[32 * h + d, :] = wk[4 * h + d]

    wv_pad = np.zeros((C + 1, 20), np.float32)
    for h in range(HEADS):
        wv_pad[C, 5 * h] = 1.0
        for d in range(DH):
            wv_pad[0:C, 5 * h + 1 + d] = wv[4 * h + d]

    wo_sp = np.zeros((128, C), np.float32)
    for h in range(HEADS):
        for d in range(DH):
            wo_sp[32 * h + 1 + d, :] = w_out[:, 4 * h + d]

    common = {
        "xa": xa_bf,
        "wq_sp": wq_sp.astype(NPB),
        "wk_sp": wk_sp.astype(NPB),
        "wv_pad": wv_pad.astype(NPB),
        "wo_sp": wo_sp.astype(NPB),
        "b_out": np.ascontiguousarray(b_out.reshape(C, 1), dtype=np.float32),
    }
    in_maps = []
    x_bf = xa_bf[:, 0:C, :]
    for c in range(NCORES):
        m = dict(common)
        m["xq"] = np.ascontiguousarray(x_bf[:, :, c * IS : (c + 1) * IS])
        in_maps.append(m)
    return in_maps


_NC_CACHE = None


def get_nc():
    global _NC_CACHE
    if _NC_CACHE is None:
        _NC_CACHE = build_graph()
    return _NC_CACHE


def run(inputs, trace=False):
    nc = get_nc()
    in_maps = host_prep(**inputs)
    res = run_bass_kernel_spmd(
        nc, in_maps, core_ids=list(range(NCORES)), trace=False
    )
    pieces_ = [res.results[c]["out"] for c in range(NCORES)]
    y = np.concatenate(pieces_, axis=2)  # [B, C, N]
    y = y.reshape(B, C, HW, HW).astype(np.float32)
    return y, res


def kernel(**inputs):
    y, _ = run(inputs, trace=False)
    return y


if __name__ == "__main__":
    rng = np.random.default_rng(0)
    ins = {
        "x": rng.standard_normal((B, C, HW, HW), dtype=np.float32),
        "w_qkv": (rng.standard_normal((48, C)) * 0.05).astype(np.float32),
        "w_out": (rng.standard_normal((C, 16)) * 0.05).astype(np.float32),
        "b_out": (rng.standard_normal(C) * 0.05).astype(np.float32),
    }
    y = kernel(**ins)
    print("out shape", y.shape, y.dtype)
